# revision 12
# baseline (speedup 1.0000x reference)
"""CAM (channel attention) module kernel for Trainium2 (Bass/Tile).

Reference computation (per batch b):
    energy  = x_b @ x_b.T                      # [C, C], contraction over N
    att     = softmax(rowmax(energy) - energy) # row-wise over last axis
    out     = att @ x_b                        # [C, N]
    y_b     = gamma * out + x_b
Identity: softmax(rowmax(E) - E)[i,j] = exp(mn[i] - E[i,j]) / Z[i],
mn[i] = min_j E[i,j] (shift invariance; exact).

Sharding: data-parallel over B across 8 NeuronCores (B=32 -> 4 per core),
gamma replicated, full CxC attention per core.

v3b design (measured-informed):
  - X arrives via GpSimd SWDGE DMA with in-flight f32->f16 cast straight
    into per-window X16 tiles (no staging, no engine cast; the v1 GpSimd
    bulk cast cost 220us/core and stalled the PE ~65us).
  - f16 (not bf16) for all 16-bit data: same PE speed, 8x the mantissa.
  - xT for matmul-1: PE transposes via *normal* matmul against an f16
    identity (1 cy/row; v1 used is_transpose on f32 data = 2 cy/row).
    One co-lane per window goes to the DMA XBAR (dma_start_transpose)
    to shave PE cycles; measured XBAR cost is ~1.25us per [128,512]
    serialized on the issuing engine, so only a slice of the work goes
    there (sync + scalar alternate).
  - tT via DMA XBAR transposes of tS (f16) -- no PE/PSUM/ScalarE cost.
  - matmul-2 f16: stationary tT chunks, moving X16 windows; evac on DVE
    (x gamma/Z + f16 residual from X16, ~2^-11 rel err).
  - Software-pipelined emission: iteration k emits mm1(k) then mm2(k-1);
    mm2(k-1) windows interleave the in-DMAs of batch k+1, so input
    windows land exactly one batch ahead of use and X16 buffers recycle
    per-window (bufs=2 per window tag).
PSUM: E 4 banks + 2 matmul-2 accumulation banks + 2 transpose banks.
"""

import contextlib

import numpy as np

P = 128

_CACHE = {}


DEFAULT_OPTS = dict(
    o_bufs=4,       # output window staging tiles [P, CO, 512] f32
    xbar_cos=(),    # co lanes whose xT transposes go to the DMA XBAR
    timing_io=False,  # x/y internal DRAM (no host transfer) -- timing runs
)


def _build(Bs, C, N, reps=1, **opts):
    import concourse.bass as bass  # noqa: F401
    import concourse.tile as tile
    import concourse.mybir as mybir
    from concourse import bacc
    from concourse.masks import make_identity

    o = dict(DEFAULT_OPTS)
    o.update(opts)

    F32 = mybir.dt.float32
    F16 = mybir.dt.float16
    AF = mybir.ActivationFunctionType
    ALU = mybir.AluOpType
    AX = mybir.AxisListType

    assert C == 4 * P and N % 512 == 0
    CO = C // P          # 4 row/col chunks of 128
    KC = N // P          # 32 contraction chunks of 128
    NW = N // 512        # 8 n-windows of 512

    xbar_cos = set(o["xbar_cos"])
    pe_cos = [co for co in range(CO) if co not in xbar_cos]

    nc = bacc.Bacc(None, target_bir_lowering=False, debug=False)
    if o["timing_io"]:
        x_in = nc.dram_tensor("x_int", [Bs, C, N], F32)
        g_in = nc.dram_tensor("gamma", [1], F32, kind="ExternalInput")
        y_out = nc.dram_tensor("y_int", [Bs, C, N], F32)
        yy_out = nc.dram_tensor("yy", [1, 1], F32, kind="ExternalOutput")
    else:
        x_in = nc.dram_tensor("x", [Bs, C, N], F32, kind="ExternalInput")
        g_in = nc.dram_tensor("gamma", [1], F32, kind="ExternalInput")
        y_out = nc.dram_tensor("y", [Bs, C, N], F32, kind="ExternalOutput")
        yy_out = None

    with tile.TileContext(nc) as tc:
        with (
            tc.tile_pool(name="consts", bufs=1) as consts,
            tc.tile_pool(name="x16p", bufs=2) as x16p,
            tc.tile_pool(name="xtp", bufs=2) as xtp,
            tc.tile_pool(name="tsp", bufs=2) as tsp,
            tc.tile_pool(name="ttp", bufs=2) as ttp,
            tc.tile_pool(name="otp", bufs=o["o_bufs"]) as otp,
            tc.tile_pool(name="stgp", bufs=2) as stgp,
            tc.tile_pool(name="stats", bufs=2) as stats,
            tc.tile_pool(name="pe", bufs=1, space="PSUM") as psum_e,
            tc.tile_pool(name="pacc", bufs=2, space="PSUM") as psum_acc,
            tc.tile_pool(name="psx", bufs=1, space="PSUM") as psum_xt,
        ):
            ident16 = consts.tile([P, P], F16)
            make_identity(nc, ident16)
            ident32 = consts.tile([P, P], F32)
            make_identity(nc, ident32)
            g_sb = consts.tile([1, 1], F32)
            nc.sync.dma_start(g_sb[:, :], g_in[:].rearrange("(a b) -> a b", a=1))
            g_col = consts.tile([P, 1], F32)
            nc.gpsimd.partition_broadcast(g_col[:, :], g_sb[:1, :1])

            if o["timing_io"]:
                zt = otp.tile([P, CO, 512], F32, tag="ot", name="zt")
                nc.gpsimd.memset(zt[:, :, :], 0.0)
                for zb in range(Bs):
                    zx = x_in[zb].rearrange("(co p) n -> p co n", p=P)
                    for zw in range(NW):
                        nc.sync.dma_start(
                            zx[:, :, zw * 512:(zw + 1) * 512], zt[:, :, :]
                        )

            st = {}  # per-batch live tiles

            def make_x16(b):
                st[b] = {
                    "X16w": [
                        x16p.tile([P, CO, 512], F16, tag=f"x16w{w}",
                                  name=f"X16w{w}")
                        for w in range(NW)
                    ]
                }

            def in_dma(b, w):
                x_b = x_in[b].rearrange("(co p) n -> p co n", p=P)
                nc.gpsimd.dma_start(
                    st[b]["X16w"][w][:, :, :],
                    x_b[:, :, w * 512:(w + 1) * 512],
                )

            def emit_mm1(b):
                X16w = st[b]["X16w"]
                xt = xtp.tile([P, KC, C], F16, tag="xt", name="xt")
                E = psum_e.tile([P, CO, C], F32, tag="E", name="E")
                mn = stats.tile([P, CO], F32, tag="mn")
                zs = stats.tile([P, CO], F32, tag="zs")
                rg = stats.tile([P, CO], F32, tag="rg")
                tS = tsp.tile([P, CO, C], F16, tag="tS")
                tT = ttp.tile([P, CO, C], F16, tag="tT")

                # XBAR transposes for the offloaded co lanes (run ahead,
                # serialized per issuing engine; sync/scalar alternate)
                for w in range(NW):
                    for i, co in enumerate(sorted(xbar_cos)):
                        eng = nc.sync if (w + i) % 2 == 0 else nc.scalar
                        eng.dma_start_transpose(
                            xt[:, w * 4:(w + 1) * 4, co * P:(co + 1) * P],
                            X16w[w][:, co, :],
                        )

                # PE transposes (normal matmul vs f16 identity) for window
                # w+1 interleave with matmul-1 chunks of window w
                def t_pe(w):
                    for i, co in enumerate(pe_cos):
                        ps = psum_xt.tile(
                            [P, 4, P], F32, tag=f"psx{(w * len(pe_cos) + i) % 2}",
                            bufs=1, name="ps_x",
                        )
                        for j in range(4):
                            nc.tensor.matmul(
                                ps[:, j, :],
                                X16w[w][:, co, j * P:(j + 1) * P],
                                ident16,
                            )
                        nc.scalar.copy(
                            xt[:, w * 4:(w + 1) * 4, co * P:(co + 1) * P],
                            ps[:, :, :],
                        )

                def mm1_chunks(w):
                    for kc in range(w * 4, (w + 1) * 4):
                        for ic in range(CO):
                            nc.tensor.matmul(
                                E[:, ic, ic * P:],
                                xt[:, kc, ic * P:(ic + 1) * P],
                                xt[:, kc, ic * P:],
                                start=(kc == 0),
                                stop=(kc == KC - 1),
                            )

                t_pe(0)
                for w in range(NW):
                    if w + 1 < NW:
                        t_pe(w + 1)
                    mm1_chunks(w)

                # mirror E[jc, ic] = E[ic, jc].T for ic < jc
                for jc in range(1, CO):
                    for ic in range(jc):
                        stg = stgp.tile([P, P], F32, tag="stg")
                        nc.scalar.copy(
                            stg[:, :], E[:, ic, jc * P:(jc + 1) * P]
                        )
                        nc.tensor.matmul(
                            E[:, jc, ic * P:(ic + 1) * P],
                            stg[:, :],
                            ident32,
                            is_transpose=True,
                            skip_group_check=True,
                        )

                # softmax: tS = exp(mn - E) in f16, Z row-sum fused (f32)
                for ic in range(CO):
                    nc.vector.tensor_reduce(
                        mn[:, ic:ic + 1], E[:, ic, :], AX.X, ALU.min
                    )
                for ic in range(CO):
                    nc.scalar.activation(
                        tS[:, ic, :], E[:, ic, :], AF.Exp,
                        bias=mn[:, ic:ic + 1], scale=-1.0,
                        accum_out=zs[:, ic:ic + 1],
                    )
                # tT[j, jc, i] via XBAR transposes of tS rows. On scalar:
                # its dep (exp) is also the last scalar op, so no false
                # ordering; sync must stay free for out-DMAs, which would
                # otherwise defer behind these and stall ot recycling.
                for ic in range(CO):
                    nc.scalar.dma_start_transpose(
                        tT[:, :, ic * P:(ic + 1) * P], tS[:, ic, :]
                    )
                st[b]["tT"] = tT
                st[b]["zs"] = zs
                st[b]["rg"] = rg

            def emit_mm2(b, prefetch_b):
                """mm2 + evac + out-DMA per n-window; interleaves the
                in-DMAs of batch `prefetch_b` (X16 recycles per window)."""
                X16w, tT, rg = st[b]["X16w"], st[b]["tT"], st[b]["rg"]
                y_b = y_out[b].rearrange("(co p) n -> p co n", p=P)
                if prefetch_b is not None:
                    make_x16(prefetch_b)
                # rg here, not in emit_mm1: on the in-order DVE a recip
                # emitted between min(b) and the evacs of mm2(b-1) would
                # block those evacs on exp(b) and stall the PE on PSUM
                nc.vector.reciprocal(rg[:, :], st[b]["zs"][:, :])
                nc.vector.tensor_scalar_mul(rg[:, :], rg[:, :], g_col[:, :1])
                for w in range(NW):
                    ot = otp.tile([P, CO, 512], F32, tag="ot")
                    for ic in range(CO):
                        ps2 = psum_acc.tile([P, 512], F32, tag="acc")
                        for jc in range(CO):
                            nc.tensor.matmul(
                                ps2[:, :],
                                tT[:, jc, ic * P:(ic + 1) * P],
                                X16w[w][:, jc, :],
                                start=(jc == 0), stop=(jc == CO - 1),
                            )
                        nc.vector.scalar_tensor_tensor(
                            ot[:, ic, :], ps2[:, :], rg[:, ic:ic + 1],
                            X16w[w][:, ic, :],
                            op0=ALU.mult, op1=ALU.add,
                        )
                    # out on sync HWDGE: SWDGE descriptors cost ~124ns/2KB
                    # vs HWDGE 17ns and would back up the DMA rings
                    nc.sync.dma_start(
                        y_b[:, :, w * 512:(w + 1) * 512], ot[:, :, :]
                    )
                    if prefetch_b is not None:
                        in_dma(prefetch_b, w)
                del st[b]

            loop_ctx = (
                tc.For_i(0, reps, 1) if reps > 1 else contextlib.nullcontext()
            )
            with loop_ctx:
                for k in range(Bs + 1):
                    if k == 0:
                        make_x16(0)
                        for w in range(NW):
                            in_dma(0, w)
                        if Bs > 1:
                            make_x16(1)
                            for w in range(NW):
                                in_dma(1, w)
                    if k < Bs:
                        emit_mm1(k)
                    if k >= 1:
                        nb = k + 1 if k + 1 < Bs else None
                        emit_mm2(k - 1, nb)

            if o["timing_io"]:
                ysb = stats.tile([1, 1], F32, tag="ysb")
                nc.sync.dma_start(
                    ysb[:1, :1], y_out[Bs - 1, C - 1:C, N - 1:N]
                )
                nc.sync.dma_start(yy_out[:1, :1], ysb[:1, :1])

    nc.compile()
    return nc


def get_nc(Bs=4, C=512, N=4096, reps=1, **opts):
    key = (Bs, C, N, reps, tuple(sorted(opts.items())))
    if key not in _CACHE:
        _CACHE[key] = _build(Bs, C, N, reps, **opts)
    return _CACHE[key]


def kernel(x, gamma):
    """Full inputs in, full output out. x [32, 512, 4096] f32, gamma [1] f32."""
    from concourse.bass_utils import run_bass_kernel_spmd

    x = np.ascontiguousarray(np.asarray(x, dtype=np.float32))
    gamma = np.ascontiguousarray(np.asarray(gamma, dtype=np.float32))
    B, C, N = x.shape
    n_cores = 8
    assert B % n_cores == 0
    Bs = B // n_cores

    nc = get_nc(Bs, C, N)
    in_maps = [
        {"x": x[i * Bs:(i + 1) * Bs], "gamma": gamma} for i in range(n_cores)
    ]
    res = run_bass_kernel_spmd(nc, in_maps, core_ids=list(range(n_cores)))
    return np.concatenate([r["y"] for r in res.results], axis=0)


# revision 14
# speedup vs baseline: 1.1053x; 1.1053x over previous
"""CAM (channel attention) module kernel for Trainium2 (Bass/Tile).

Reference computation (per batch b):
    energy  = x_b @ x_b.T                      # [C, C], contraction over N
    att     = softmax(rowmax(energy) - energy) # row-wise over last axis
    out     = att @ x_b                        # [C, N]
    y_b     = gamma * out + x_b
Identity: softmax(rowmax(E) - E)[i,j] = exp(mn[i] - E[i,j]) / Z[i],
mn[i] = min_j E[i,j] (shift invariance; exact).

Sharding: data-parallel over B across 8 NeuronCores (B=32 -> 4 per core),
gamma replicated, full CxC attention per core.

v3b design (measured-informed):
  - X arrives via GpSimd SWDGE DMA with in-flight f32->f16 cast straight
    into per-window X16 tiles (no staging, no engine cast; the v1 GpSimd
    bulk cast cost 220us/core and stalled the PE ~65us).
  - f16 (not bf16) for all 16-bit data: same PE speed, 8x the mantissa.
  - xT for matmul-1: PE transposes via *normal* matmul against an f16
    identity (1 cy/row; v1 used is_transpose on f32 data = 2 cy/row).
    One co-lane per window goes to the DMA XBAR (dma_start_transpose)
    to shave PE cycles; measured XBAR cost is ~1.25us per [128,512]
    serialized on the issuing engine, so only a slice of the work goes
    there (sync + scalar alternate).
  - tT via DMA XBAR transposes of tS (f16) -- no PE/PSUM/ScalarE cost.
  - matmul-2 f16: stationary tT chunks, moving X16 windows; evac on DVE
    (x gamma/Z + f16 residual from X16, ~2^-11 rel err).
  - Software-pipelined emission: iteration k emits mm1(k) then mm2(k-1);
    mm2(k-1) windows interleave the in-DMAs of batch k+1, so input
    windows land exactly one batch ahead of use and X16 buffers recycle
    per-window (bufs=2 per window tag).
PSUM: E 4 banks + 2 matmul-2 accumulation banks + 2 transpose banks.
"""

import contextlib

import numpy as np

P = 128

_CACHE = {}


DEFAULT_OPTS = dict(
    o_bufs=4,       # output window staging tiles [P, CO, 512] f32
    xbar_cos=(),    # co lanes whose xT transposes go to the DMA XBAR
    timing_io=False,  # x/y internal DRAM (no host transfer) -- timing runs
)


def _build(Bs, C, N, reps=1, **opts):
    import concourse.bass as bass  # noqa: F401
    import concourse.tile as tile
    import concourse.mybir as mybir
    from concourse import bacc
    from concourse.masks import make_identity

    o = dict(DEFAULT_OPTS)
    o.update(opts)

    F32 = mybir.dt.float32
    F16 = mybir.dt.float16
    AF = mybir.ActivationFunctionType
    ALU = mybir.AluOpType
    AX = mybir.AxisListType

    assert C == 4 * P and N % 512 == 0
    CO = C // P          # 4 row/col chunks of 128
    KC = N // P          # 32 contraction chunks of 128
    NW = N // 512        # 8 n-windows of 512

    xbar_cos = set(o["xbar_cos"])
    pe_cos = [co for co in range(CO) if co not in xbar_cos]

    nc = bacc.Bacc(None, target_bir_lowering=False, debug=False)
    if o["timing_io"]:
        x_in = nc.dram_tensor("x_int", [Bs, C, N], F32)
        g_in = nc.dram_tensor("gamma", [1], F32, kind="ExternalInput")
        y_out = nc.dram_tensor("y_int", [Bs, C, N], F32)
        yy_out = nc.dram_tensor("yy", [1, 1], F32, kind="ExternalOutput")
    else:
        x_in = nc.dram_tensor("x", [Bs, C, N], F32, kind="ExternalInput")
        g_in = nc.dram_tensor("gamma", [1], F32, kind="ExternalInput")
        y_out = nc.dram_tensor("y", [Bs, C, N], F32, kind="ExternalOutput")
        yy_out = None

    with tile.TileContext(nc) as tc:
        with (
            tc.tile_pool(name="consts", bufs=1) as consts,
            tc.tile_pool(name="x16p", bufs=2) as x16p,
            tc.tile_pool(name="xtp", bufs=2) as xtp,
            tc.tile_pool(name="tsp", bufs=2) as tsp,
            tc.tile_pool(name="ttp", bufs=2) as ttp,
            tc.tile_pool(name="otp", bufs=o["o_bufs"]) as otp,
            tc.tile_pool(name="stgp", bufs=2) as stgp,
            tc.tile_pool(name="stats", bufs=2) as stats,
            tc.tile_pool(name="pe", bufs=1, space="PSUM") as psum_e,
            tc.tile_pool(name="pacc", bufs=2, space="PSUM") as psum_acc,
            tc.tile_pool(name="psx", bufs=1, space="PSUM") as psum_xt,
        ):
            ident16 = consts.tile([P, P], F16)
            make_identity(nc, ident16)
            ident32 = consts.tile([P, P], F32)
            make_identity(nc, ident32)
            g_sb = consts.tile([1, 1], F32)
            nc.sync.dma_start(g_sb[:, :], g_in[:].rearrange("(a b) -> a b", a=1))
            g_col = consts.tile([P, 1], F32)
            nc.gpsimd.partition_broadcast(g_col[:, :], g_sb[:1, :1])

            if o["timing_io"]:
                zt = otp.tile([P, CO, 512], F32, tag="ot", name="zt")
                nc.gpsimd.memset(zt[:, :, :], 0.0)
                for zb in range(Bs):
                    zx = x_in[zb].rearrange("(co p) n -> p co n", p=P)
                    for zw in range(NW):
                        nc.sync.dma_start(
                            zx[:, :, zw * 512:(zw + 1) * 512], zt[:, :, :]
                        )

            st = {}  # per-batch live tiles

            # X16 stored in 1024-wide double-windows: halves the SWDGE
            # descriptor count of the casting in-DMAs (~124ns/descriptor
            # on the rings) vs 512-wide windows
            ND = NW // 2

            def make_x16(b):
                st[b] = {
                    "X16w": [
                        x16p.tile([P, CO, 1024], F16, tag=f"x16w{w}",
                                  name=f"X16w{w}")
                        for w in range(ND)
                    ]
                }

            def x16_slice(b, nf):
                """[P, CO, 512] view of n-window nf (0..NW-1)."""
                h = nf % 2
                return st[b]["X16w"][nf // 2][:, :, h * 512:(h + 1) * 512]

            def in_dma(b, w):
                x_b = x_in[b].rearrange("(co p) n -> p co n", p=P)
                nc.gpsimd.dma_start(
                    st[b]["X16w"][w][:, :, :],
                    x_b[:, :, w * 1024:(w + 1) * 1024],
                )

            def emit_mm1(b):
                Xs = lambda w: x16_slice(b, w)  # noqa: E731
                xt = xtp.tile([P, KC, C], F16, tag="xt", name="xt")
                E = psum_e.tile([P, CO, C], F32, tag="E", name="E")
                mn = stats.tile([P, CO], F32, tag="mn")
                zs = stats.tile([P, CO], F32, tag="zs")
                rg = stats.tile([P, CO], F32, tag="rg")
                tS = tsp.tile([P, CO, C], F16, tag="tS")
                tT = ttp.tile([P, CO, C], F16, tag="tT")

                # XBAR transposes for the offloaded co lanes (run ahead,
                # serialized per issuing engine; sync/scalar alternate)
                for w in range(NW):
                    for i, co in enumerate(sorted(xbar_cos)):
                        eng = nc.sync if (w + i) % 2 == 0 else nc.scalar
                        eng.dma_start_transpose(
                            xt[:, w * 4:(w + 1) * 4, co * P:(co + 1) * P],
                            Xs(w)[:, co, :],
                        )

                # PE transposes (normal matmul vs f16 identity) for window
                # w+1 interleave with matmul-1 chunks of window w
                def t_pe(w):
                    for i, co in enumerate(pe_cos):
                        ps = psum_xt.tile(
                            [P, 4, P], F32, tag=f"psx{(w * len(pe_cos) + i) % 2}",
                            bufs=1, name="ps_x",
                        )
                        for j in range(4):
                            nc.tensor.matmul(
                                ps[:, j, :],
                                Xs(w)[:, co, j * P:(j + 1) * P],
                                ident16,
                            )
                        nc.scalar.copy(
                            xt[:, w * 4:(w + 1) * 4, co * P:(co + 1) * P],
                            ps[:, :, :],
                        )

                def mm1_chunks(w):
                    for kc in range(w * 4, (w + 1) * 4):
                        for ic in range(CO):
                            nc.tensor.matmul(
                                E[:, ic, ic * P:],
                                xt[:, kc, ic * P:(ic + 1) * P],
                                xt[:, kc, ic * P:],
                                start=(kc == 0),
                                stop=(kc == KC - 1),
                            )

                t_pe(0)
                for w in range(NW):
                    if w + 1 < NW:
                        t_pe(w + 1)
                    mm1_chunks(w)

                # mirror E[jc, ic] = E[ic, jc].T for ic < jc
                for jc in range(1, CO):
                    for ic in range(jc):
                        stg = stgp.tile([P, P], F32, tag="stg")
                        nc.scalar.copy(
                            stg[:, :], E[:, ic, jc * P:(jc + 1) * P]
                        )
                        nc.tensor.matmul(
                            E[:, jc, ic * P:(ic + 1) * P],
                            stg[:, :],
                            ident32,
                            is_transpose=True,
                            skip_group_check=True,
                        )

                # softmax: tS = exp(mn - E) in f16, Z row-sum fused (f32)
                for ic in range(CO):
                    nc.vector.tensor_reduce(
                        mn[:, ic:ic + 1], E[:, ic, :], AX.X, ALU.min
                    )
                for ic in range(CO):
                    nc.scalar.activation(
                        tS[:, ic, :], E[:, ic, :], AF.Exp,
                        bias=mn[:, ic:ic + 1], scale=-1.0,
                        accum_out=zs[:, ic:ic + 1],
                    )
                # tT[j, jc, i] via XBAR transposes of tS rows. On scalar:
                # its dep (exp) is also the last scalar op, so no false
                # ordering; sync must stay free for out-DMAs, which would
                # otherwise defer behind these and stall ot recycling.
                for ic in range(CO):
                    nc.scalar.dma_start_transpose(
                        tT[:, :, ic * P:(ic + 1) * P], tS[:, ic, :]
                    )
                st[b]["tT"] = tT
                st[b]["zs"] = zs
                st[b]["rg"] = rg

            def emit_mm2(b, prefetch_b):
                """mm2 + evac + out-DMA per n-window; interleaves the
                in-DMAs of batch `prefetch_b` (X16 recycles per window)."""
                tT, rg = st[b]["tT"], st[b]["rg"]
                y_b = y_out[b].rearrange("(co p) n -> p co n", p=P)
                if prefetch_b is not None:
                    make_x16(prefetch_b)
                # rg here, not in emit_mm1: on the in-order DVE a recip
                # emitted between min(b) and the evacs of mm2(b-1) would
                # block those evacs on exp(b) and stall the PE on PSUM
                nc.vector.reciprocal(rg[:, :], st[b]["zs"][:, :])
                nc.vector.tensor_scalar_mul(rg[:, :], rg[:, :], g_col[:, :1])
                for w in range(NW):
                    ot = otp.tile([P, CO, 512], F32, tag="ot")
                    for ic in range(CO):
                        ps2 = psum_acc.tile([P, 512], F32, tag="acc")
                        for jc in range(CO):
                            nc.tensor.matmul(
                                ps2[:, :],
                                tT[:, jc, ic * P:(ic + 1) * P],
                                x16_slice(b, w)[:, jc, :],
                                start=(jc == 0), stop=(jc == CO - 1),
                            )
                        nc.vector.scalar_tensor_tensor(
                            ot[:, ic, :], ps2[:, :], rg[:, ic:ic + 1],
                            x16_slice(b, w)[:, ic, :],
                            op0=ALU.mult, op1=ALU.add,
                        )
                    # out on sync HWDGE: SWDGE descriptors cost ~124ns/2KB
                    # vs HWDGE 17ns and would back up the DMA rings
                    nc.sync.dma_start(
                        y_b[:, :, w * 512:(w + 1) * 512], ot[:, :, :]
                    )
                    if prefetch_b is not None and w % 2 == 1:
                        in_dma(prefetch_b, w // 2)
                del st[b]

            loop_ctx = (
                tc.For_i(0, reps, 1) if reps > 1 else contextlib.nullcontext()
            )
            with loop_ctx:
                for k in range(Bs + 1):
                    if k == 0:
                        make_x16(0)
                        for w in range(ND):
                            in_dma(0, w)
                        if Bs > 1:
                            make_x16(1)
                            for w in range(ND):
                                in_dma(1, w)
                    if k < Bs:
                        emit_mm1(k)
                    if k >= 1:
                        nb = k + 1 if k + 1 < Bs else None
                        emit_mm2(k - 1, nb)

            if o["timing_io"]:
                ysb = stats.tile([1, 1], F32, tag="ysb")
                nc.sync.dma_start(
                    ysb[:1, :1], y_out[Bs - 1, C - 1:C, N - 1:N]
                )
                nc.sync.dma_start(yy_out[:1, :1], ysb[:1, :1])

    nc.compile()
    return nc


def get_nc(Bs=4, C=512, N=4096, reps=1, **opts):
    key = (Bs, C, N, reps, tuple(sorted(opts.items())))
    if key not in _CACHE:
        _CACHE[key] = _build(Bs, C, N, reps, **opts)
    return _CACHE[key]


def kernel(x, gamma):
    """Full inputs in, full output out. x [32, 512, 4096] f32, gamma [1] f32."""
    from concourse.bass_utils import run_bass_kernel_spmd

    x = np.ascontiguousarray(np.asarray(x, dtype=np.float32))
    gamma = np.ascontiguousarray(np.asarray(gamma, dtype=np.float32))
    B, C, N = x.shape
    n_cores = 8
    assert B % n_cores == 0
    Bs = B // n_cores

    nc = get_nc(Bs, C, N)
    in_maps = [
        {"x": x[i * Bs:(i + 1) * Bs], "gamma": gamma} for i in range(n_cores)
    ]
    res = run_bass_kernel_spmd(nc, in_maps, core_ids=list(range(n_cores)))
    return np.concatenate([r["y"] for r in res.results], axis=0)
